# revision 38
# baseline (speedup 1.0000x reference)
"""Multi-head graph attention kernel for Trainium2, SPMD over 8 NeuronCores.

Sharding: core c owns batch b=c//4 and destination-row slice
i in [512*(c%4), 512*(c%4+1)), all 8 heads.  Each core computes complete
softmax rows (j ranges over all 2048 sources); the host concatenates the
per-core [512, 256] output slabs.  No cross-core collectives.

Score math (per core, layout [j=partition, i=free]).  Softmax rows are
invariant to any per-i scale, so divide the classic GAT score by
B_i = exp(e_src_i):
  s'_ji = P2_ji * max(A_j, C_j * r_i)
     A=exp(e_dst), C=exp(alpha*e_dst), r=exp((alpha-1)*e_src)
  P2_ji = (priorM_ji + eps)^beta,  priorM = prior*adj  (host-masked; the
     eps^beta leakage on non-edges is ~2.6e-4 of a typical row mass)
  u = (Rb*C) max A  -- ONE fused vector tensor_scalar (two scalar APs)
  s = u * P2        -- one vector tensor_tensor
  hT[f,i] = sum_j xp[j,f] * s_ji  (PE; a ones-column gives Z_i)
  out[i,:] = (hT[:,i]/Z) @ W_out.T
e_dst rides the projection matmul via Wext = [Wall | wdT] (264 cols).
"""

import math
import sys
from contextlib import ExitStack

sys.path.insert(0, "/opt/trn_rl_repo")

import numpy as np
import ml_dtypes

import concourse.bass as bass
import concourse.tile as tile
from concourse import bacc, mybir
from concourse.bass_utils import run_bass_kernel_spmd

B, N, D, H = 2, 2048, 256, 8
DH = D // H          # 32
NC = 8
ISL = N // 4         # 512 destination rows per core
NJ = N // 128        # 16 j-tiles
EPS = 1e-6
ALPHA = 0.2

F32 = mybir.dt.float32
BF16 = mybir.dt.bfloat16

AF = mybir.ActivationFunctionType
OP = mybir.AluOpType

BF = ml_dtypes.bfloat16

_cache = {}
last_run_info = {}


def _build(beta: float):
    nc = bacc.Bacc(
        "TRN2",
        target_bir_lowering=False,
        debug=False,
        enable_asserts=False,
        num_devices=NC,
    )

    def inp(name, shape, dt):
        return nc.dram_tensor(name, shape, dt, kind="ExternalInput").ap()

    xT_d = inp("xT", [D, N], BF16)         # x[b].T
    xTs_d = inp("xTs", [D, ISL], BF16)     # x[b, i_slice].T
    prT_d = inp("prT", [N, ISL], F32)      # (prior*adj)[b, i_slice, :].T
    # [W head-major cols | (W@a_dst).T | alpha*(W@a_dst).T] -- the two e_dst
    # column groups let A=exp(e) and C=exp(alpha*e) use the SAME Exp table
    Wext_d = inp("Wext", [D, D + 2 * H], BF16)
    WoT_d = inp("WoT", [D, D], BF16)       # W_out.T
    wsT_d = inp("wsT", [D, H], BF16)       # (W@a_src per head).T
    sel_d = inp("sel", [8, 2 * 128], BF16)  # 1/Z band-broadcast selectors
    oneh_d = inp("oneh", [8, H * 128], BF16)  # row-h-of-rr broadcast selectors
    out_d = nc.dram_tensor("out", [ISL, D], F32, kind="ExternalOutput").ap()

    with tile.TileContext(nc) as tc, ExitStack() as ctx:
        pp = ctx.enter_context(tc.tile_pool(name="persist", bufs=1))
        wk = ctx.enter_context(tc.tile_pool(name="work", bufs=3))

        # ---- resident inputs (all bf16 except prior)
        xT = [pp.tile([128, N], BF16, tag=f"xT{k}", name=f"xT{k}") for k in range(2)]
        xTs = [pp.tile([128, ISL], BF16, tag=f"xTs{k}", name=f"xTs{k}") for k in range(2)]
        Wext = [pp.tile([128, D + 2 * H], BF16, tag=f"We{k}", name=f"We{k}") for k in range(2)]
        WoT = [pp.tile([128, D], BF16, tag=f"WoT{k}", name=f"WoT{k}") for k in range(2)]
        wsT = [pp.tile([128, H], BF16, tag=f"wsT{k}", name=f"wsT{k}") for k in range(2)]
        # DMA order: the small tensors gating the critical chain (es -> rr
        # -> Rb, Ln0 -> P2[0], first projection) go first; bulk xT/prior after
        prts = []
        for k in range(2):
            r = slice(k * 128, (k + 1) * 128)
            nc.sync.dma_start(xTs[k][:], xTs_d[r, :])
            nc.sync.dma_start(wsT[k][:], wsT_d[r, :])
            nc.sync.dma_start(Wext[k][:], Wext_d[r, :])
        prt = wk.tile([128, ISL], F32, tag="prt", name="prt", bufs=4)
        nc.sync.dma_start(prt[:], prT_d[0:128, :])
        prts.append(prt)
        for k in range(2):
            r = slice(k * 128, (k + 1) * 128)
            nc.sync.dma_start(xT[k][:], xT_d[r, :])
            nc.sync.dma_start(WoT[k][:], WoT_d[r, :])
        for jt in range(1, NJ):
            r = slice(jt * 128, (jt + 1) * 128)
            prt = wk.tile([128, ISL], F32, tag="prt", name="prt", bufs=4)
            nc.sync.dma_start(prt[:], prT_d[r, :])
            prts.append(prt)

        ones1s = pp.tile([1, 128], BF16, tag="ones1s", name="ones1s")
        nc.vector.memset(ones1s[:], 1.0)
        epsb = pp.tile([128, 1], F32, tag="epsb", name="epsb")
        nc.vector.memset(epsb[:], EPS)
        # selector matrices for the 1/Z band broadcast: sel[k][h, p] = 1
        # iff h == 4k + p//32 (host constant; compute engines can't write
        # partition offsets that aren't multiples of 32)
        sel = pp.tile([8, 2 * 128], BF16, tag="sel", name="sel")
        nc.sync.dma_start(sel[:], sel_d[:, :])
        oneh = pp.tile([8, H * 128], BF16, tag="oneh", name="oneh")
        nc.sync.dma_start(oneh[:], oneh_d[:, :])

        # ---- persistent intermediates
        xp_aug = pp.tile([128, NJ * H * 33], BF16, tag="xpaug", name="xpaug")
        nc.vector.memset(xp_aug[:], 1.0)  # ones col per 33-block survives
        AC_t = pp.tile([128, NJ * 2 * H], F32, tag="ACt", name="ACt")
        Rb = pp.tile([128, H * ISL], BF16, tag="Rb", name="Rb")
        lnp = pp.tile([128, NJ * ISL], F32, tag="lnp", name="lnp")
        P2 = pp.tile([128, NJ * ISL], BF16, tag="P2", name="P2")
        hcat = [pp.tile([128, ISL], BF16, tag=f"hcat{k}", name=f"hcat{k}") for k in range(2)]
        zall = pp.tile([8, ISL], F32, tag="zall", name="zall")

        # ================= phase 1: projections (+e_dst), e_src, broadcasts
        with tc.tile_pool(name="ps1", bufs=2, space="PSUM") as ps1:
            # e_src rows for all heads at once: [8, ISL]
            es_ps = ps1.tile([8, ISL], F32, tag="es", name="es")
            for k in range(2):
                nc.tensor.matmul(
                    es_ps[:], wsT[k][:], xTs[k][:],
                    start=(k == 0), stop=(k == 1),
                )
            # rr first on the scalar queue: it gates the Rb broadcasts that
            # the whole u-op stream depends on
            rr = pp.tile([8, ISL], BF16, tag="rr", name="rr")
            nc.scalar.activation(rr[:], es_ps[:], AF.Exp, scale=ALPHA - 1.0)
            # P2 for jt=0 as early as possible (gates the first s-multiply)
            ci0 = slice(0, ISL)
            nc.scalar.activation(lnp[:, ci0], prts[0][:], AF.Ln, bias=epsb[:])
            nc.scalar.activation(P2[:, ci0], lnp[:, ci0], AF.Exp, scale=beta)
            for h in range(H):
                rb_ps = ps1.tile([128, ISL], F32, tag="rb", name="rb")
                nc.tensor.matmul(
                    rb_ps[:], oneh[:, h * 128:(h + 1) * 128], rr[:],
                    start=True, stop=True,
                )
                nc.vector.tensor_copy(Rb[:, h * ISL:(h + 1) * ISL], rb_ps[:])

            for jt in range(NJ):
                c = slice(jt * 128, (jt + 1) * 128)
                xp_ps = ps1.tile([128, D + 2 * H], F32, tag="xp", name="xp",
                                 bufs=4)
                for k in range(2):
                    nc.tensor.matmul(
                        xp_ps[:], xT[k][:, c], Wext[k][:],
                        start=(k == 0), stop=(k == 1),
                    )
                dst = (
                    xp_aug[:, jt * 264:(jt + 1) * 264]
                    .rearrange("p (h w) -> p h w", w=33)[:, :, 0:32]
                )
                src = xp_ps[:, 0:D].rearrange("p (h w) -> p h w", w=32)
                # split the PSUM->SBUF drain across vector and scalar
                if jt % 2 == 0:
                    nc.vector.tensor_copy(dst, src)
                else:
                    nc.scalar.copy(dst, src)
                cj = slice(jt * 16, (jt + 1) * 16)
                nc.scalar.activation(AC_t[:, cj], xp_ps[:, D:D + 2 * H],
                                     AF.Exp)

        # ================= phase 2a: P2 from prior (scalar engine only)
        # jt=0 was handled early above; the rest go in blocks of 4 per
        # activation table so P2[jt] is ready just-in-time for the jt loop.
        for blk in range(1, 5):
            jts = range(1 + 4 * (blk - 1), min(1 + 4 * blk, NJ))
            for jt in jts:
                ci = slice(jt * ISL, (jt + 1) * ISL)
                nc.scalar.activation(lnp[:, ci], prts[jt][:], AF.Ln,
                                     bias=epsb[:])
            for jt in jts:
                ci = slice(jt * ISL, (jt + 1) * ISL)
                nc.scalar.activation(P2[:, ci], lnp[:, ci], AF.Exp, scale=beta)

        # ================= phase 2b: scores + attention (jt-outer)
        with tc.tile_pool(name="ps2", bufs=1, space="PSUM") as ps2:
            hT_ps = [ps2.tile([33, ISL], F32, tag=f"hT{h}", name=f"hT{h}")
                     for h in range(H)]
            for jt in range(NJ):
                ci = slice(jt * ISL, (jt + 1) * ISL)
                for h in range(H):
                    ch = slice(h * ISL, (h + 1) * ISL)
                    cc = slice(jt * 16 + 8 + h, jt * 16 + 8 + h + 1)
                    ca = slice(jt * 16 + h, jt * 16 + h + 1)
                    u = wk.tile([128, ISL], BF16, tag="u", name="u", bufs=12)
                    nc.vector.tensor_scalar(
                        u[:], Rb[:, ch], AC_t[:, cc], AC_t[:, ca],
                        OP.mult, OP.max,
                    )
                    s = wk.tile([128, ISL], BF16, tag="s", name="s", bufs=12)
                    nc.vector.tensor_tensor(s[:], u[:], P2[:, ci], OP.mult)
                    lw = slice(jt * 264 + h * 33, jt * 264 + (h + 1) * 33)
                    nc.tensor.matmul(
                        hT_ps[h][:], xp_aug[:, lw], s[:],
                        start=(jt == 0), stop=(jt == NJ - 1),
                    )
                    if jt == NJ - 1:
                        # drain head h immediately after its last matmul so
                        # the tail overlaps the remaining heads' matmuls:
                        # Z row (PSUM p32 -> SBUF p0 -> DMA to zall row h)
                        # + raw (unnormalized) head output to SBUF
                        zrow = wk.tile([1, ISL], F32, tag="zrow",
                                       name="zrow", bufs=2)
                        nc.scalar.copy(zrow[:], hT_ps[h][32:33, :])
                        nc.sync.dma_start(zall[h:h + 1, :], zrow[:])
                        ph = slice((h % 4) * 32, (h % 4) * 32 + 32)
                        if h % 2 == 0:
                            nc.vector.tensor_copy(hcat[h // 4][ph, :],
                                                  hT_ps[h][0:32, :])
                        else:
                            nc.scalar.copy(hcat[h // 4][ph, :],
                                           hT_ps[h][0:32, :])

        # ================= phase 3: normalize + output projection
        zinv = pp.tile([8, ISL], F32, tag="zinv", name="zinv")
        zsc = pp.tile([8, ISL], F32, tag="zsc", name="zsc")
        nc.vector.reciprocal_approx_accurate(zinv[:], zall[:], zsc[:])
        zinv_c = pp.tile([8, ISL], BF16, tag="zinvc", name="zinvc")
        nc.vector.tensor_copy(zinv_c[:], zinv[:])
        with tc.tile_pool(name="ps3", bufs=1, space="PSUM") as ps3:
            hn = [pp.tile([128, ISL], BF16, tag=f"hn{k}", name=f"hn{k}")
                  for k in range(2)]
            for k in range(2):
                zb_ps = ps3.tile([128, ISL], F32, tag="zb", name="zb", bufs=2)
                nc.tensor.matmul(
                    zb_ps[:], sel[:, k * 128:(k + 1) * 128], zinv_c[:],
                    start=True, stop=True,
                )
                nc.vector.tensor_tensor(hn[k][:], hcat[k][:], zb_ps[:], OP.mult)
            for ic in range(4):
                cc = slice(ic * 128, (ic + 1) * 128)
                op_ps = ps3.tile([128, D], F32, tag="op", name="op", bufs=2)
                for k in range(2):
                    nc.tensor.matmul(
                        op_ps[:], hn[k][:, cc], WoT[k][:],
                        start=(k == 0), stop=(k == 1),
                    )
                ob = wk.tile([128, D], F32, tag="ob", name="ob")
                nc.scalar.copy(ob[:], op_ps[:])
                nc.sync.dma_start(out_d[cc, :], ob[:])

    nc.compile()
    return nc


def _get_program(beta: float):
    key = round(beta, 9)
    if key not in _cache:
        _cache[key] = _build(beta)
    return _cache[key]


def kernel(x, adj, prior, W, a_src, a_dst, beta_tilde, W_out, **kw):
    global last_run_info
    x = np.asarray(x, np.float32)
    adj = np.asarray(adj)
    prior = np.asarray(prior, np.float32)
    W = np.asarray(W, np.float32)
    a_src = np.asarray(a_src, np.float32)
    a_dst = np.asarray(a_dst, np.float32)
    W_out = np.asarray(W_out, np.float32)
    assert x.shape == (B, N, D) and prior.shape == (B, N, N)

    bt = float(np.asarray(beta_tilde))
    beta = float(math.log1p(math.exp(bt)))
    nc = _get_program(beta)

    xT = np.ascontiguousarray(x.transpose(0, 2, 1))               # [B, D, N]
    Wall = np.ascontiguousarray(W.transpose(1, 0, 2).reshape(D, D))
    wdT = np.einsum("hdf,hf->hd", W, a_dst).T                     # [D, H]
    Wext = np.concatenate([Wall, wdT, ALPHA * wdT], axis=1).astype(BF)
    WoT = np.ascontiguousarray(W_out.T).astype(BF)
    wsT = np.ascontiguousarray(np.einsum("hdf,hf->hd", W, a_src).T).astype(BF)
    priorM = prior * adj.astype(np.float32)[None, :, :]           # host mask
    sel = np.zeros((8, 256), BF)
    for k in range(2):
        for q in range(4):
            sel[4 * k + q, k * 128 + q * 32:k * 128 + (q + 1) * 32] = 1.0
    oneh = np.zeros((8, 8 * 128), BF)
    for h in range(8):
        oneh[h, h * 128:(h + 1) * 128] = 1.0

    xT_bf = xT.astype(BF)
    in_maps = []
    for c in range(NC):
        b, q = c // 4, c % 4
        i0 = q * ISL
        in_maps.append({
            "xT": xT_bf[b],
            "xTs": np.ascontiguousarray(xT_bf[b][:, i0:i0 + ISL]),
            "prT": np.ascontiguousarray(priorM[b, i0:i0 + ISL, :].T),
            "Wext": Wext,
            "WoT": WoT,
            "wsT": wsT,
            "sel": sel,
            "oneh": oneh,
        })

    trace = bool(kw.get("trace", False))
    res = run_bass_kernel_spmd(
        nc, in_maps, core_ids=list(range(NC)), trace=trace
    )
    last_run_info = {
        "exec_time_ns": res.exec_time_ns,
        "mean_exec_time_ns": res.mean_exec_time_ns,
        "trace": res.instructions_and_trace[1]
        if res.instructions_and_trace else None,
    }

    out = np.empty((B, N, D), np.float32)
    for c in range(NC):
        b, q = c // 4, c % 4
        out[b, q * ISL:(q + 1) * ISL, :] = res.results[c]["out"]
    return out


# revision 39
# speedup vs baseline: 1.0018x; 1.0018x over previous
"""Multi-head graph attention kernel for Trainium2, SPMD over 8 NeuronCores.

Sharding: core c owns batch b=c//4 and destination-row slice
i in [512*(c%4), 512*(c%4+1)), all 8 heads.  Each core computes complete
softmax rows (j ranges over all 2048 sources); the host concatenates the
per-core [512, 256] output slabs.  No cross-core collectives.

Score math (per core, layout [j=partition, i=free]).  Softmax rows are
invariant to any per-i scale, so divide the classic GAT score by
B_i = exp(e_src_i):
  s'_ji = P2_ji * max(A_j, C_j * r_i)
     A=exp(e_dst), C=exp(alpha*e_dst), r=exp((alpha-1)*e_src)
  P2_ji = (priorM_ji + eps)^beta,  priorM = prior*adj  (host-masked; the
     eps^beta leakage on non-edges is ~2.6e-4 of a typical row mass)
  u = (Rb*C) max A  -- ONE fused vector tensor_scalar (two scalar APs)
  s = u * P2        -- one vector tensor_tensor
  hT[f,i] = sum_j xp[j,f] * s_ji  (PE; a ones-column gives Z_i)
  out[i,:] = (hT[:,i]/Z) @ W_out.T
e_dst rides the projection matmul via Wext = [Wall | wdT] (264 cols).
"""

import math
import sys
from contextlib import ExitStack

sys.path.insert(0, "/opt/trn_rl_repo")

import numpy as np
import ml_dtypes

import concourse.bass as bass
import concourse.tile as tile
from concourse import bacc, mybir
from concourse.bass_utils import run_bass_kernel_spmd

B, N, D, H = 2, 2048, 256, 8
DH = D // H          # 32
NC = 8
ISL = N // 4         # 512 destination rows per core
NJ = N // 128        # 16 j-tiles
EPS = 1e-6
ALPHA = 0.2

F32 = mybir.dt.float32
BF16 = mybir.dt.bfloat16

AF = mybir.ActivationFunctionType
OP = mybir.AluOpType

BF = ml_dtypes.bfloat16

_cache = {}
last_run_info = {}


def _build(beta: float):
    nc = bacc.Bacc(
        "TRN2",
        target_bir_lowering=False,
        debug=False,
        enable_asserts=False,
        num_devices=NC,
    )

    def inp(name, shape, dt):
        return nc.dram_tensor(name, shape, dt, kind="ExternalInput").ap()

    xT_d = inp("xT", [D, N], BF16)         # x[b].T
    xTs_d = inp("xTs", [D, ISL], BF16)     # x[b, i_slice].T
    prT_d = inp("prT", [N, ISL], F32)      # (prior*adj)[b, i_slice, :].T
    # [W head-major cols | (W@a_dst).T | alpha*(W@a_dst).T] -- the two e_dst
    # column groups let A=exp(e) and C=exp(alpha*e) use the SAME Exp table
    Wext_d = inp("Wext", [D, D + 2 * H], BF16)
    WoT_d = inp("WoT", [D, D], BF16)       # W_out.T
    wsT_d = inp("wsT", [D, H], BF16)       # (W@a_src per head).T
    sel_d = inp("sel", [8, 2 * 128], BF16)  # 1/Z band-broadcast selectors
    oneh_d = inp("oneh", [8, H * 128], BF16)  # row-h-of-rr broadcast selectors
    out_d = nc.dram_tensor("out", [ISL, D], F32, kind="ExternalOutput").ap()

    with tile.TileContext(nc) as tc, ExitStack() as ctx:
        pp = ctx.enter_context(tc.tile_pool(name="persist", bufs=1))
        wk = ctx.enter_context(tc.tile_pool(name="work", bufs=3))

        # ---- resident inputs (all bf16 except prior)
        xT = [pp.tile([128, N], BF16, tag=f"xT{k}", name=f"xT{k}") for k in range(2)]
        xTs = [pp.tile([128, ISL], BF16, tag=f"xTs{k}", name=f"xTs{k}") for k in range(2)]
        Wext = [pp.tile([128, D + 2 * H], BF16, tag=f"We{k}", name=f"We{k}") for k in range(2)]
        WoT = [pp.tile([128, D], BF16, tag=f"WoT{k}", name=f"WoT{k}") for k in range(2)]
        wsT = [pp.tile([128, H], BF16, tag=f"wsT{k}", name=f"wsT{k}") for k in range(2)]
        # DMA order: the small tensors gating the critical chain (es -> rr
        # -> Rb, Ln0 -> P2[0], first projection) go first; bulk xT/prior after
        prts = []
        for k in range(2):
            r = slice(k * 128, (k + 1) * 128)
            nc.sync.dma_start(xTs[k][:], xTs_d[r, :])
            nc.sync.dma_start(wsT[k][:], wsT_d[r, :])
            nc.sync.dma_start(Wext[k][:], Wext_d[r, :])
        prt = wk.tile([128, ISL], F32, tag="prt", name="prt", bufs=4)
        nc.sync.dma_start(prt[:], prT_d[0:128, :])
        prts.append(prt)
        for k in range(2):
            r = slice(k * 128, (k + 1) * 128)
            nc.sync.dma_start(xT[k][:], xT_d[r, :])
            nc.sync.dma_start(WoT[k][:], WoT_d[r, :])
        for jt in range(1, NJ):
            r = slice(jt * 128, (jt + 1) * 128)
            prt = wk.tile([128, ISL], F32, tag="prt", name="prt", bufs=4)
            nc.sync.dma_start(prt[:], prT_d[r, :])
            prts.append(prt)

        ones1s = pp.tile([1, 128], BF16, tag="ones1s", name="ones1s")
        nc.vector.memset(ones1s[:], 1.0)
        epsb = pp.tile([128, 1], F32, tag="epsb", name="epsb")
        nc.vector.memset(epsb[:], EPS)
        # selector matrices for the 1/Z band broadcast: sel[k][h, p] = 1
        # iff h == 4k + p//32 (host constant; compute engines can't write
        # partition offsets that aren't multiples of 32)
        sel = pp.tile([8, 2 * 128], BF16, tag="sel", name="sel")
        nc.sync.dma_start(sel[:], sel_d[:, :])
        oneh = pp.tile([8, H * 128], BF16, tag="oneh", name="oneh")
        nc.sync.dma_start(oneh[:], oneh_d[:, :])

        # ---- persistent intermediates
        xp_aug = pp.tile([128, NJ * H * 33], BF16, tag="xpaug", name="xpaug")
        nc.vector.memset(xp_aug[:], 1.0)  # ones col per 33-block survives
        AC_t = pp.tile([128, NJ * 2 * H], F32, tag="ACt", name="ACt")
        Rb = pp.tile([128, H * ISL], BF16, tag="Rb", name="Rb")
        lnp = pp.tile([128, NJ * ISL], F32, tag="lnp", name="lnp")
        P2 = pp.tile([128, NJ * ISL], BF16, tag="P2", name="P2")
        hcat = [pp.tile([128, ISL], BF16, tag=f"hcat{k}", name=f"hcat{k}") for k in range(2)]
        zall = pp.tile([8, ISL], F32, tag="zall", name="zall")

        # ================= phase 1: projections (+e_dst), e_src, broadcasts
        with tc.tile_pool(name="ps1", bufs=2, space="PSUM") as ps1:
            # e_src rows for all heads at once: [8, ISL]
            es_ps = ps1.tile([8, ISL], F32, tag="es", name="es")
            for k in range(2):
                nc.tensor.matmul(
                    es_ps[:], wsT[k][:], xTs[k][:],
                    start=(k == 0), stop=(k == 1),
                )
            # P2 for jt=0 as early as possible (gates the first s-multiply)
            ci0 = slice(0, ISL)
            nc.scalar.activation(lnp[:, ci0], prts[0][:], AF.Ln, bias=epsb[:])
            nc.scalar.activation(P2[:, ci0], lnp[:, ci0], AF.Exp, scale=beta)
            rr = pp.tile([8, ISL], BF16, tag="rr", name="rr")
            nc.scalar.activation(rr[:], es_ps[:], AF.Exp, scale=ALPHA - 1.0)
            for h in range(H):
                rb_ps = ps1.tile([128, ISL], F32, tag="rb", name="rb")
                nc.tensor.matmul(
                    rb_ps[:], oneh[:, h * 128:(h + 1) * 128], rr[:],
                    start=True, stop=True,
                )
                nc.vector.tensor_copy(Rb[:, h * ISL:(h + 1) * ISL], rb_ps[:])

            for jt in range(NJ):
                c = slice(jt * 128, (jt + 1) * 128)
                xp_ps = ps1.tile([128, D + 2 * H], F32, tag="xp", name="xp",
                                 bufs=4)
                for k in range(2):
                    nc.tensor.matmul(
                        xp_ps[:], xT[k][:, c], Wext[k][:],
                        start=(k == 0), stop=(k == 1),
                    )
                dst = (
                    xp_aug[:, jt * 264:(jt + 1) * 264]
                    .rearrange("p (h w) -> p h w", w=33)[:, :, 0:32]
                )
                src = xp_ps[:, 0:D].rearrange("p (h w) -> p h w", w=32)
                # split the PSUM->SBUF drain across vector and scalar
                if jt % 2 == 0:
                    nc.vector.tensor_copy(dst, src)
                else:
                    nc.scalar.copy(dst, src)
                cj = slice(jt * 16, (jt + 1) * 16)
                nc.scalar.activation(AC_t[:, cj], xp_ps[:, D:D + 2 * H],
                                     AF.Exp)

        # ================= phase 2a: P2 from prior (scalar engine only)
        # jt=0 was handled early above; the rest go in blocks of 4 per
        # activation table so P2[jt] is ready just-in-time for the jt loop.
        for blk in range(1, 5):
            jts = range(1 + 4 * (blk - 1), min(1 + 4 * blk, NJ))
            for jt in jts:
                ci = slice(jt * ISL, (jt + 1) * ISL)
                nc.scalar.activation(lnp[:, ci], prts[jt][:], AF.Ln,
                                     bias=epsb[:])
            for jt in jts:
                ci = slice(jt * ISL, (jt + 1) * ISL)
                nc.scalar.activation(P2[:, ci], lnp[:, ci], AF.Exp, scale=beta)

        # ================= phase 2b: scores + attention (jt-outer)
        with tc.tile_pool(name="ps2", bufs=1, space="PSUM") as ps2:
            hT_ps = [ps2.tile([33, ISL], F32, tag=f"hT{h}", name=f"hT{h}")
                     for h in range(H)]
            for jt in range(NJ):
                ci = slice(jt * ISL, (jt + 1) * ISL)
                for h in range(H):
                    ch = slice(h * ISL, (h + 1) * ISL)
                    cc = slice(jt * 16 + 8 + h, jt * 16 + 8 + h + 1)
                    ca = slice(jt * 16 + h, jt * 16 + h + 1)
                    u = wk.tile([128, ISL], BF16, tag="u", name="u", bufs=12)
                    nc.vector.tensor_scalar(
                        u[:], Rb[:, ch], AC_t[:, cc], AC_t[:, ca],
                        OP.mult, OP.max,
                    )
                    s = wk.tile([128, ISL], BF16, tag="s", name="s", bufs=12)
                    nc.vector.tensor_tensor(s[:], u[:], P2[:, ci], OP.mult)
                    lw = slice(jt * 264 + h * 33, jt * 264 + (h + 1) * 33)
                    nc.tensor.matmul(
                        hT_ps[h][:], xp_aug[:, lw], s[:],
                        start=(jt == 0), stop=(jt == NJ - 1),
                    )
                    if jt == NJ - 1:
                        # drain head h immediately after its last matmul so
                        # the tail overlaps the remaining heads' matmuls:
                        # Z row (PSUM p32 -> SBUF p0 -> DMA to zall row h)
                        # + raw (unnormalized) head output to SBUF
                        zrow = wk.tile([1, ISL], F32, tag="zrow",
                                       name="zrow", bufs=2)
                        nc.scalar.copy(zrow[:], hT_ps[h][32:33, :])
                        nc.sync.dma_start(zall[h:h + 1, :], zrow[:])
                        ph = slice((h % 4) * 32, (h % 4) * 32 + 32)
                        if h % 2 == 0:
                            nc.vector.tensor_copy(hcat[h // 4][ph, :],
                                                  hT_ps[h][0:32, :])
                        else:
                            nc.scalar.copy(hcat[h // 4][ph, :],
                                           hT_ps[h][0:32, :])

        # ================= phase 3: normalize + output projection
        zinv = pp.tile([8, ISL], F32, tag="zinv", name="zinv")
        zsc = pp.tile([8, ISL], F32, tag="zsc", name="zsc")
        nc.vector.reciprocal_approx_accurate(zinv[:], zall[:], zsc[:])
        zinv_c = pp.tile([8, ISL], BF16, tag="zinvc", name="zinvc")
        nc.vector.tensor_copy(zinv_c[:], zinv[:])
        with tc.tile_pool(name="ps3", bufs=1, space="PSUM") as ps3:
            hn = [pp.tile([128, ISL], BF16, tag=f"hn{k}", name=f"hn{k}")
                  for k in range(2)]
            for k in range(2):
                zb_ps = ps3.tile([128, ISL], F32, tag="zb", name="zb", bufs=2)
                nc.tensor.matmul(
                    zb_ps[:], sel[:, k * 128:(k + 1) * 128], zinv_c[:],
                    start=True, stop=True,
                )
                nc.vector.tensor_tensor(hn[k][:], hcat[k][:], zb_ps[:], OP.mult)
            for ic in range(4):
                cc = slice(ic * 128, (ic + 1) * 128)
                op_ps = ps3.tile([128, D], F32, tag="op", name="op", bufs=2)
                for k in range(2):
                    nc.tensor.matmul(
                        op_ps[:], hn[k][:, cc], WoT[k][:],
                        start=(k == 0), stop=(k == 1),
                    )
                ob = wk.tile([128, D], F32, tag="ob", name="ob")
                nc.scalar.copy(ob[:], op_ps[:])
                nc.sync.dma_start(out_d[cc, :], ob[:])

    nc.compile()
    return nc


def _get_program(beta: float):
    key = round(beta, 9)
    if key not in _cache:
        _cache[key] = _build(beta)
    return _cache[key]


def kernel(x, adj, prior, W, a_src, a_dst, beta_tilde, W_out, **kw):
    global last_run_info
    x = np.asarray(x, np.float32)
    adj = np.asarray(adj)
    prior = np.asarray(prior, np.float32)
    W = np.asarray(W, np.float32)
    a_src = np.asarray(a_src, np.float32)
    a_dst = np.asarray(a_dst, np.float32)
    W_out = np.asarray(W_out, np.float32)
    assert x.shape == (B, N, D) and prior.shape == (B, N, N)

    bt = float(np.asarray(beta_tilde))
    beta = float(math.log1p(math.exp(bt)))
    nc = _get_program(beta)

    xT = np.ascontiguousarray(x.transpose(0, 2, 1))               # [B, D, N]
    Wall = np.ascontiguousarray(W.transpose(1, 0, 2).reshape(D, D))
    wdT = np.einsum("hdf,hf->hd", W, a_dst).T                     # [D, H]
    Wext = np.concatenate([Wall, wdT, ALPHA * wdT], axis=1).astype(BF)
    WoT = np.ascontiguousarray(W_out.T).astype(BF)
    wsT = np.ascontiguousarray(np.einsum("hdf,hf->hd", W, a_src).T).astype(BF)
    priorM = prior * adj.astype(np.float32)[None, :, :]           # host mask
    sel = np.zeros((8, 256), BF)
    for k in range(2):
        for q in range(4):
            sel[4 * k + q, k * 128 + q * 32:k * 128 + (q + 1) * 32] = 1.0
    oneh = np.zeros((8, 8 * 128), BF)
    for h in range(8):
        oneh[h, h * 128:(h + 1) * 128] = 1.0

    xT_bf = xT.astype(BF)
    in_maps = []
    for c in range(NC):
        b, q = c // 4, c % 4
        i0 = q * ISL
        in_maps.append({
            "xT": xT_bf[b],
            "xTs": np.ascontiguousarray(xT_bf[b][:, i0:i0 + ISL]),
            "prT": np.ascontiguousarray(priorM[b, i0:i0 + ISL, :].T),
            "Wext": Wext,
            "WoT": WoT,
            "wsT": wsT,
            "sel": sel,
            "oneh": oneh,
        })

    trace = bool(kw.get("trace", False))
    res = run_bass_kernel_spmd(
        nc, in_maps, core_ids=list(range(NC)), trace=trace
    )
    last_run_info = {
        "exec_time_ns": res.exec_time_ns,
        "mean_exec_time_ns": res.mean_exec_time_ns,
        "trace": res.instructions_and_trace[1]
        if res.instructions_and_trace else None,
    }

    out = np.empty((B, N, D), np.float32)
    for c in range(NC):
        b, q = c // 4, c % 4
        out[b, q * ISL:(q + 1) * ISL, :] = res.results[c]["out"]
    return out
